# revision 27
# baseline (speedup 1.0000x reference)
"""Bass/Trainium2 kernel for nn_CharLevelLanguageModel (6-layer char transformer).

Strategy: data-parallel over batch (64 -> 8 cores x 8). Full forward pass in one
NEFF per core. Activations live feature-major (x_T [C, tokens]) in SBUF as
float32r (TF32-like matmul dtype, 1 cy/row on the PE at N>=256). LayerNorm
gains/biases are folded into the adjacent weights on the host, so on-device LN
is pure standardization done with ones-matmul stats + gpsimd partition
broadcasts. Attention is computed key-major (S_T[k,q]) so softmax needs no
transposes: exp on ACT, sums via an appended ones-column in V, normalization
via reciprocal + partition_broadcast. Causal masking is two additive -1e9
triangular constants accumulated into the score PSUM by identity matmuls.
"""

import os
import numpy as np

import concourse.bass as bass
import concourse.mybir as mybir
import concourse.tile as tile
from concourse import bacc
from concourse.bass_utils import run_bass_kernel_spmd
from concourse.masks import make_identity

B, T, C, H, L, V = 64, 256, 384, 6, 6, 65
HS = C // H          # 64
DFF = 4 * C          # 1536
N_CORES = 8
BPC = B // N_CORES   # 8 batches per core
NTOK = BPC * T       # 2048 tokens per core
NT = NTOK // 512     # 4 column tiles of 512
KC = C // 128        # 3 feature chunks
EPS = 1e-5
SCALE = HS ** -0.5

f32 = mybir.dt.float32
f32r = mybir.dt.float32r
AF = mybir.ActivationFunctionType
ALU = mybir.AluOpType

N_LAYERS = int(os.environ.get("KERNEL_LAYERS", str(L)))

_cache = {}


def _build_nc():
    nc = bacc.Bacc("TRN2", target_bir_lowering=False, debug=False,
                   num_devices=N_CORES)

    # ---- DRAM parameters ----
    x0T_d = nc.dram_tensor("x0T", [C, NTOK], f32r, kind="ExternalInput").ap()
    wqkv_d = nc.dram_tensor("wqkv", [L, C, 3 * C], f32r, kind="ExternalInput").ap()
    bqkv_d = nc.dram_tensor("bqkv", [L, 2 * C], f32, kind="ExternalInput").ap()
    wo_d = nc.dram_tensor("wo", [L, C, C], f32r, kind="ExternalInput").ap()
    bo_d = nc.dram_tensor("bo", [L, C], f32, kind="ExternalInput").ap()
    w1_d = nc.dram_tensor("w1", [L, C, DFF], f32r, kind="ExternalInput").ap()
    b1_d = nc.dram_tensor("b1", [L, DFF], f32, kind="ExternalInput").ap()
    w2_d = nc.dram_tensor("w2", [L, DFF, C], f32r, kind="ExternalInput").ap()
    b2_d = nc.dram_tensor("b2", [L, C], f32, kind="ExternalInput").ap()
    wlm_d = nc.dram_tensor("wlm", [C, V], f32r, kind="ExternalInput").ap()
    wqksum_d = nc.dram_tensor("wqksum", [L, 2, 2 * C], f32r, kind="ExternalInput").ap()
    wvsum_d = nc.dram_tensor("wvsum", [L, 2, C], f32r, kind="ExternalInput").ap()
    w1sum_d = nc.dram_tensor("w1sum", [L, 2, DFF], f32r, kind="ExternalInput").ap()
    wlmsum_d = nc.dram_tensor("wlmsum", [2, V], f32r, kind="ExternalInput").ap()
    blm_d = nc.dram_tensor("blm", [V], f32, kind="ExternalInput").ap()
    outT_d = nc.dram_tensor("outT", [V, NTOK], f32, kind="ExternalOutput").ap()

    with tile.TileContext(nc) as tc:
        _build_body(nc, tc, x0T_d, wqkv_d, bqkv_d, wo_d, bo_d, w1_d, b1_d,
                    w2_d, b2_d, wlm_d, blm_d, outT_d,
                    wqksum_d, wvsum_d, w1sum_d, wlmsum_d)
    nc.compile()
    return nc


def _build_body(nc, tc, x0T_d, wqkv_d, bqkv_d, wo_d, bo_d, w1_d, b1_d,
                w2_d, b2_d, wlm_d, blm_d, outT_d,
                wqksum_d, wvsum_d, w1sum_d, wlmsum_d):
    import contextlib
    ctx = contextlib.ExitStack()
    p_const = ctx.enter_context(tc.tile_pool(name="consts", bufs=1))
    p_x = ctx.enter_context(tc.tile_pool(name="x", bufs=1))
    p_xn = ctx.enter_context(tc.tile_pool(name="xn", bufs=1))
    p_qk = ctx.enter_context(tc.tile_pool(name="qk", bufs=2))
    p_v = ctx.enter_context(tc.tile_pool(name="v", bufs=2))
    p_w = ctx.enter_context(tc.tile_pool(name="w", bufs=1))
    p_b = ctx.enter_context(tc.tile_pool(name="b", bufs=2))
    p_tr = ctx.enter_context(tc.tile_pool(name="tr", bufs=2))   # transient 512-wide
    p_at = ctx.enter_context(tc.tile_pool(name="at", bufs=2))   # attc accum
    p_sm = ctx.enter_context(tc.tile_pool(name="sm", bufs=2))   # rec / r_b
    p_e = ctx.enter_context(tc.tile_pool(name="e", bufs=2))
    p_st = ctx.enter_context(tc.tile_pool(name="st", bufs=2))   # LN stats rows
    p_bc = ctx.enter_context(tc.tile_pool(name="bc", bufs=2))   # LN broadcast tiles
    p_ff = ctx.enter_context(tc.tile_pool(name="ff", bufs=2))
    p_out = ctx.enter_context(tc.tile_pool(name="out", bufs=1))
    psum = ctx.enter_context(tc.tile_pool(name="psum", bufs=2, space="PSUM"))

    # ---- constants ----
    stage = p_const.tile([128, 64], f32, tag="stage")

    # m01 [128,512] 0/1 multiplicative causal mask: cols 0:256 keep p<=f
    # (kb0), cols 256:512 keep p<=f-256 i.e. local q offset 128 (kb1)
    m01 = p_const.tile([128, 512], f32, tag="m01")
    nc.vector.memset(m01[:, 0:256], 1.0)
    nc.gpsimd.affine_select(out=m01[:, 0:256], in_=m01[:, 0:256],
                            compare_op=ALU.is_ge, fill=0.0,
                            base=0, pattern=[[1, 256]], channel_multiplier=-1)
    nc.vector.memset(m01[:, 256:512], 1.0)
    nc.gpsimd.affine_select(out=m01[:, 256:512], in_=m01[:, 256:512],
                            compare_op=ALU.is_ge, fill=0.0,
                            base=-128, pattern=[[1, 256]], channel_multiplier=-1)

    onesC = p_const.tile([128, 2], f32r, tag="onesC")   # 1/C for mean matmuls
    nc.vector.memset(stage[:, 0:2], 1.0 / C)
    nc.vector.tensor_copy(onesC[:], stage[:, 0:2])
    onesH = p_const.tile([128, H], f32r, tag="onesH")  # ones columns for V_ext
    nc.vector.memset(stage[:, 8:8 + H], 1.0)
    nc.vector.tensor_copy(onesH[:], stage[:, 8:8 + H])
    eps_t = p_const.tile([1, 1], f32, tag="eps")
    nc.vector.memset(eps_t, EPS)

    blm_t = p_const.tile([V, 1], f32, tag="blm")
    nc.sync.dma_start(out=blm_t, in_=blm_d.rearrange("(v o) -> v o", o=1))
    wlm_t = [p_const.tile([128, V], f32r, tag=f"wlm{kc}", name=f"wlm{kc}")
             for kc in range(KC)]
    for kc in range(KC):
        nc.sync.dma_start(out=wlm_t[kc], in_=wlm_d[kc * 128:(kc + 1) * 128, :])

    # ---- residual stream ----
    x_t = [[p_x.tile([128, 512], f32r, tag=f"x{kc}_{nt}", name=f"x{kc}_{nt}")
            for nt in range(NT)] for kc in range(KC)]
    for kc in range(KC):
        for nt in range(NT):
            nc.sync.dma_start(out=x_t[kc][nt],
                              in_=x0T_d[kc * 128:(kc + 1) * 128,
                                        nt * 512:nt * 512 + 512])

    def layernorm(src_tiles, tagp):
        """scale-standardize: returns xs = x * rs_b as a 3x4 grid of f32r
        tiles; the mean correction mr = mu*rs is left in mr2 row 0 and must
        be folded by the consumer via a K=2 rank-1 matmul with -colsum
        weights (xn = xs - broadcast(mr))."""
        out_tiles = [[p_xn.tile([128, 512], f32r, tag=f"{tagp}{kc}_{nt}",
                                name=f"{tagp}{kc}_{nt}") for nt in range(NT)]
                     for kc in range(KC)]
        for nt in range(NT):
            cols = slice(nt * 512, nt * 512 + 512)
            c0 = nt * 512
            A = p_st.tile([1, 512], f32, tag="lnA", name="lnA")[:]
            Br = p_st.tile([1, 512], f32, tag="lnB", name="lnB")[:]
            Cr = p_st.tile([1, 512], f32, tag="lnC", name="lnC")[:]
            mu_ps = psum.tile([2, 512], f32, tag="pa", name="mu_ps")
            sq_ps = psum.tile([2, 512], f32, tag="pb", name="sq_ps")
            for kc in range(KC):
                nc.tensor.matmul(mu_ps[:], onesC[:], src_tiles[kc][nt][:],
                                 start=(kc == 0), stop=(kc == KC - 1))
            for kc in range(KC):
                xsq = p_tr.tile([128, 512], f32r, tag="tr512", name="xsq")
                nc.scalar.square(xsq[:], src_tiles[kc][nt][:])
                nc.tensor.matmul(sq_ps[:], onesC[:], xsq[:],
                                 start=(kc == 0), stop=(kc == KC - 1))
            nc.scalar.copy(A, mu_ps[0:1, :])
            nc.scalar.copy(Br, sq_ps[0:1, :])
            nc.vector.tensor_mul(Cr, A, A)                       # tmp = mu^2
            nc.vector.tensor_tensor(out=Br, in0=Br, in1=Cr, op=ALU.subtract)
            nc.scalar.activation(Br, Br, AF.Sqrt, bias=eps_t[:], scale=1.0)
            nc.vector.reciprocal_approx_fast(out=Br, in_=Br)     # B = rs
            nc.vector.tensor_mul(Cr, A, Br)                      # C = mr = mu*rs
            rs_b = p_bc.tile([128, 512], f32, tag="rs_b")
            mr_b = p_bc.tile([128, 512], f32, tag="mr_b")
            nc.gpsimd.partition_broadcast(rs_b[:], Br)
            nc.gpsimd.partition_broadcast(mr_b[:], Cr)
            for kc in range(KC):
                t1 = p_tr.tile([128, 512], f32r, tag="tr512", name="lnt1")
                nc.gpsimd.tensor_mul(t1[:], src_tiles[kc][nt][:], rs_b[:])
                nc.vector.tensor_tensor(out=out_tiles[kc][nt][:], in0=t1[:],
                                        in1=mr_b[:], op=ALU.subtract)
        return out_tiles

    for l in range(N_LAYERS):
        # ---- per-layer weights ----
        wqkv_t = [p_w.tile([128, 3 * C], f32r, tag=f"wqkv{kc}", name=f"wqkv{kc}")
                  for kc in range(KC)]
        for kc in range(KC):
            nc.sync.dma_start(out=wqkv_t[kc],
                              in_=wqkv_d[l, kc * 128:(kc + 1) * 128, :])
        bqkv_t = p_b.tile([128, 6], f32, tag="bqkv")
        nc.sync.dma_start(out=bqkv_t,
                          in_=bqkv_d[l].rearrange("(a p) -> p a", p=128))
        wo_t = [p_w.tile([128, C], f32r, tag=f"wo{kc}", name=f"wo{kc}")
                for kc in range(KC)]
        for kc in range(KC):
            nc.sync.dma_start(out=wo_t[kc], in_=wo_d[l, kc * 128:(kc + 1) * 128, :])
        bo_t = p_b.tile([128, 3], f32, tag="bo")
        nc.sync.dma_start(out=bo_t, in_=bo_d[l].rearrange("(a p) -> p a", p=128))
        w1_t = [p_w.tile([128, DFF], f32r, tag=f"w1{kc}", name=f"w1{kc}")
                for kc in range(KC)]
        for kc in range(KC):
            nc.sync.dma_start(out=w1_t[kc], in_=w1_d[l, kc * 128:(kc + 1) * 128, :])
        b1_t = p_b.tile([128, 12], f32, tag="b1")
        nc.sync.dma_start(out=b1_t, in_=b1_d[l].rearrange("(a p) -> p a", p=128))
        w2_t = [p_w.tile([128, C], f32r, tag=f"w2{kc}", name=f"w2k{kc}")
                for kc in range(12)]
        for kc in range(12):
            nc.sync.dma_start(out=w2_t[kc], in_=w2_d[l, kc * 128:(kc + 1) * 128, :])
        b2_t = p_b.tile([128, 3], f32, tag="b2")
        nc.sync.dma_start(out=b2_t, in_=b2_d[l].rearrange("(a p) -> p a", p=128))

        # ---- LN1 ----
        xn = layernorm(x_t, "ln")

        # ---- attention, per pair of batches (QKV/Wo at N=512) ----
        for bp in range(BPC // 2):
            nt = bp  # 512-col tile index == batch pair index
            # Q,K for 2 batches: feature-major [128, 512]
            qk_t = [p_qk.tile([128, 512], f32r, tag=f"qk{oc}", name=f"qk{oc}")
                    for oc in range(6)]
            for oc in range(6):
                qp = psum.tile([128, 512], f32, tag="pa", name="qp")
                for kc in range(KC):
                    nc.tensor.matmul(qp[:], wqkv_t[kc][:, oc * 128:oc * 128 + 128],
                                     xn[kc][nt][:],
                                     start=(kc == 0), stop=(kc == KC - 1))
                nc.scalar.activation(qk_t[oc][:], qp[:], AF.Identity,
                                     bias=bqkv_t[:, oc:oc + 1], scale=1.0)
            attc = [p_at.tile([128, 512], f32r, tag=f"attc{kc}", name=f"attc{kc}")
                    for kc in range(KC)]
            for bi in range(2):
                q0 = bi * 256        # local q offset in the pair
                # V token-major with ones column, 2 chunks of 128 tokens
                vext = [p_v.tile([128, H * (HS + 1)], f32r, tag=f"vext{i}",
                                 name=f"vext{i}") for i in range(2)]
                for i in range(2):
                    vp = psum.tile([128, C], f32, tag="pb", name="vp")
                    tc0 = q0 + i * 128
                    for kc in range(KC):
                        nc.tensor.matmul(vp[:], xn[kc][nt][:, tc0:tc0 + 128],
                                         wqkv_t[kc][:, 2 * C:3 * C],
                                         start=(kc == 0), stop=(kc == KC - 1))
                    vx = vext[i].rearrange("p (h e) -> p h e", h=H)
                    nc.vector.tensor_copy(vx[:, :, 0:HS],
                                          vp[:].rearrange("p (h d) -> p h d", h=H))
                    nc.gpsimd.tensor_copy(out=vx[:, :, HS:HS + 1], in_=onesH[:])

                for h in range(H):
                    qrow = (h % 2) * 64
                    qch, kch = h // 2, 3 + h // 2
                    sp = psum.tile([128, 512], f32, tag="pc", name="sp")
                    qs = qk_t[qch][qrow:qrow + 64, q0:q0 + 256]
                    nc.tensor.matmul(sp[:, 0:256],
                                     qk_t[kch][qrow:qrow + 64, q0:q0 + 128],
                                     qs, start=True, stop=True)
                    nc.tensor.matmul(sp[:, 256:512],
                                     qk_t[kch][qrow:qrow + 64, q0 + 128:q0 + 256],
                                     qs, start=True, stop=True)
                    e_t = p_e.tile([128, 512], f32, tag="e")
                    nc.scalar.activation(e_t[:], sp[:], AF.Exp, bias=0.0,
                                         scale=SCALE)
                    e_m = p_e.tile([128, 512], f32r, tag="em")
                    nc.vector.tensor_mul(e_m[:], e_t[:], m01[:])
                    ap_ = psum.tile([HS + 1, T], f32, tag="pd", name="ap_")
                    nc.tensor.matmul(ap_[:],
                                     vext[0][:, h * (HS + 1):(h + 1) * (HS + 1)],
                                     e_m[:, 0:256], start=True, stop=False)
                    nc.tensor.matmul(ap_[:],
                                     vext[1][:, h * (HS + 1):(h + 1) * (HS + 1)],
                                     e_m[:, 256:512], start=False, stop=True)
                    srow = p_sm.tile([1, T], f32, tag="srow")
                    nc.scalar.copy(srow[:], ap_[HS:HS + 1, :])
                    rec = p_sm.tile([1, T], f32, tag="rec")
                    nc.vector.reciprocal_approx_fast(out=rec[:], in_=srow[:])
                    r_b = p_sm.tile([64, T], f32, tag="r_b")
                    nc.gpsimd.partition_broadcast(r_b[:], rec[:])
                    nc.vector.tensor_mul(
                        attc[h // 2][qrow:qrow + 64, q0:q0 + 256],
                        ap_[0:HS, :], r_b[:])

            # Wo + residual for this batch pair (N=512)
            for oc in range(KC):
                wp = psum.tile([128, 512], f32, tag="pa", name="wp")
                for kc in range(KC):
                    nc.tensor.matmul(wp[:], wo_t[kc][:, oc * 128:oc * 128 + 128],
                                     attc[kc][:], start=(kc == 0),
                                     stop=(kc == KC - 1))
                wsb = p_tr.tile([128, 512], f32, tag="tr512", name="wsb")
                nc.scalar.activation(wsb[:], wp[:], AF.Identity,
                                     bias=bo_t[:, oc:oc + 1], scale=1.0)
                nc.gpsimd.tensor_add(x_t[oc][nt][:], wsb[:], x_t[oc][nt][:])

        # ---- LN2 + FFN (interleaved: each ff1 chunk consumed right away) ----
        h2 = layernorm(x_t, "ln")
        for nt in range(NT):
            cols = slice(nt * 512, nt * 512 + 512)
            fp2 = [psum.tile([128, 512], f32, tag=t, name=f"fp2{t}")
                   for t in ("pa", "pb", "pc")]
            for kc12 in range(12):
                fp1 = psum.tile([128, 512], f32, tag="pd", name="fp1")
                for kc in range(KC):
                    nc.tensor.matmul(fp1[:], w1_t[kc][:, kc12 * 128:kc12 * 128 + 128],
                                     h2[kc][nt][:],
                                     start=(kc == 0), stop=(kc == KC - 1))
                ff1 = p_ff.tile([128, 512], f32r, tag="ff1", name="ff1")
                nc.scalar.activation(ff1[:], fp1[:], AF.Relu,
                                     bias=b1_t[:, kc12:kc12 + 1], scale=1.0)
                for oc in range(KC):
                    nc.tensor.matmul(fp2[oc][:], w2_t[kc12][:, oc * 128:oc * 128 + 128],
                                     ff1[:], start=(kc12 == 0), stop=(kc12 == 11))
            for oc in range(KC):
                fsb = p_tr.tile([128, 512], f32, tag="tr512", name="fsb")
                nc.scalar.activation(fsb[:], fp2[oc][:], AF.Identity,
                                     bias=b2_t[:, oc:oc + 1], scale=1.0)
                nc.vector.tensor_add(x_t[oc][nt][:], fsb[:], x_t[oc][nt][:])

    # ---- final LN + LM head ----
    xf = layernorm(x_t, "ln")
    for nt in range(NT):
        cols = slice(nt * 512, nt * 512 + 512)
        lp = psum.tile([V, 512], f32, tag="pa", name="lp")
        for kc in range(KC):
            nc.tensor.matmul(lp[:], wlm_t[kc][:], xf[kc][nt][:],
                             start=(kc == 0), stop=(kc == KC - 1))
        osb = p_out.tile([V, 512], f32, tag="osb")
        nc.scalar.activation(osb[:], lp[:], AF.Identity, bias=blm_t[:], scale=1.0)
        nc.sync.dma_start(out=outT_d[:, cols], in_=osb[:])

    ctx.close()


def _host_prep(inputs):
    """Fold LN affine params into weights; build per-core input maps."""
    f = lambda k: np.asarray(inputs[k], dtype=np.float32)
    idx = np.asarray(inputs["idx"]).astype(np.int64)
    tok_emb, pos_emb = f("tok_emb"), f("pos_emb")
    Wq, Wk, Wv, Wo = f("Wq"), f("Wk"), f("Wv"), f("Wo")
    bo, W1, b1, W2, b2 = f("bo"), f("W1"), f("b1"), f("W2"), f("b2")
    ln1_g, ln1_b = f("ln1_g"), f("ln1_b")
    ln2_g, ln2_b = f("ln2_g"), f("ln2_b")
    lnf_g, lnf_b = f("lnf_g"), f("lnf_b")
    Wlm, blm = f("Wlm"), f("blm")

    # [L,H,C,HS] -> [L,C,H*HS]
    Wq_all = np.transpose(Wq, (0, 2, 1, 3)).reshape(L, C, C)
    Wk_all = np.transpose(Wk, (0, 2, 1, 3)).reshape(L, C, C)
    Wv_all = np.transpose(Wv, (0, 2, 1, 3)).reshape(L, C, C)

    g1 = ln1_g[:, :, None]
    wqkv = np.concatenate([g1 * Wq_all, g1 * Wk_all, g1 * Wv_all], axis=2)
    def neg_colsum2(w):                      # [L?,C,D] -> [.,2,D] row0=-colsum
        s = -w.sum(axis=-2)
        z = np.zeros_like(s)
        return np.stack([s, z], axis=-2)
    bq = np.einsum("lc,lcd->ld", ln1_b, Wq_all)
    bk = np.einsum("lc,lcd->ld", ln1_b, Wk_all)
    bv = np.einsum("lc,lcd->ld", ln1_b, Wv_all)
    bqkv = np.concatenate([bq, bk], axis=1)
    bo2 = bo + np.einsum("ld,ldc->lc", bv, Wo)       # v-bias folds through Wo
    w1f = ln2_g[:, :, None] * W1
    b1f = b1 + np.einsum("lc,lcd->ld", ln2_b, W1)
    wlmf = lnf_g[:, None] * Wlm
    blmf = blm + lnf_b @ Wlm

    wqksum = neg_colsum2(wqkv[:, :, :2 * C])         # [L,2,768]
    wvsum = neg_colsum2(wqkv[:, :, 2 * C:])          # [L,2,384]
    w1sum = neg_colsum2(w1f)                         # [L,2,1536]
    wlmsum = neg_colsum2(wlmf)                       # [2,65]

    x0 = tok_emb[idx] + pos_emb[None]                # [B,T,C] f32
    in_maps = []
    for c in range(N_CORES):
        x0c = x0[c * BPC:(c + 1) * BPC].reshape(NTOK, C)
        in_maps.append({
            "x0T": np.ascontiguousarray(x0c.T),
            "wqkv": np.ascontiguousarray(wqkv),
            "bqkv": np.ascontiguousarray(bqkv),
            "wo": np.ascontiguousarray(Wo),
            "bo": np.ascontiguousarray(bo2),
            "w1": np.ascontiguousarray(w1f),
            "b1": np.ascontiguousarray(b1f),
            "w2": np.ascontiguousarray(W2),
            "b2": np.ascontiguousarray(b2),
            "wlm": np.ascontiguousarray(wlmf),
            "blm": np.ascontiguousarray(blmf),
            "wqksum": np.ascontiguousarray(wqksum),
            "wvsum": np.ascontiguousarray(wvsum),
            "w1sum": np.ascontiguousarray(w1sum),
            "wlmsum": np.ascontiguousarray(wlmsum),
        })
    return in_maps


def _run(inputs, trace=False):
    if "nc" not in _cache:
        _cache["nc"] = _build_nc()
    nc = _cache["nc"]
    in_maps = _host_prep(inputs)
    res = run_bass_kernel_spmd(nc, in_maps, core_ids=list(range(N_CORES)),
                               trace=trace)
    outs = []
    for c in range(N_CORES):
        outT = res.results[c]["outT"]                 # [V, NTOK]
        outs.append(outT.T.reshape(BPC, T, V))
    logits = np.concatenate(outs, axis=0).astype(np.float32)
    return logits, res


def kernel(**inputs) -> np.ndarray:
    logits, _ = _run(inputs, trace=False)
    return logits


# revision 28
# speedup vs baseline: 1.0317x; 1.0317x over previous
"""Bass/Trainium2 kernel for nn_CharLevelLanguageModel (6-layer char transformer).

Strategy: data-parallel over batch (64 -> 8 cores x 8). Full forward pass in one
NEFF per core. Activations live feature-major (x_T [C, tokens]) in SBUF as
float32r (TF32-like matmul dtype, 1 cy/row on the PE at N>=256). LayerNorm
gains/biases are folded into the adjacent weights on the host, so on-device LN
is pure standardization done with ones-matmul stats + gpsimd partition
broadcasts. Attention is computed key-major (S_T[k,q]) so softmax needs no
transposes: exp on ACT, sums via an appended ones-column in V, normalization
via reciprocal + partition_broadcast. Causal masking is two additive -1e9
triangular constants accumulated into the score PSUM by identity matmuls.
"""

import os
import numpy as np

import concourse.bass as bass
import concourse.mybir as mybir
import concourse.tile as tile
from concourse import bacc
from concourse.bass_utils import run_bass_kernel_spmd
from concourse.masks import make_identity

B, T, C, H, L, V = 64, 256, 384, 6, 6, 65
HS = C // H          # 64
DFF = 4 * C          # 1536
N_CORES = 8
BPC = B // N_CORES   # 8 batches per core
NTOK = BPC * T       # 2048 tokens per core
NT = NTOK // 512     # 4 column tiles of 512
KC = C // 128        # 3 feature chunks
EPS = 1e-5
SCALE = HS ** -0.5

f32 = mybir.dt.float32
f32r = mybir.dt.float32r
AF = mybir.ActivationFunctionType
ALU = mybir.AluOpType

N_LAYERS = int(os.environ.get("KERNEL_LAYERS", str(L)))

_cache = {}


def _build_nc():
    nc = bacc.Bacc("TRN2", target_bir_lowering=False, debug=False,
                   num_devices=N_CORES)

    # ---- DRAM parameters ----
    x0T_d = nc.dram_tensor("x0T", [C, NTOK], f32r, kind="ExternalInput").ap()
    wqkv_d = nc.dram_tensor("wqkv", [L, C, 3 * C], f32r, kind="ExternalInput").ap()
    bqkv_d = nc.dram_tensor("bqkv", [L, 2 * C], f32, kind="ExternalInput").ap()
    wo_d = nc.dram_tensor("wo", [L, C, C], f32r, kind="ExternalInput").ap()
    bo_d = nc.dram_tensor("bo", [L, C], f32, kind="ExternalInput").ap()
    w1_d = nc.dram_tensor("w1", [L, C, DFF], f32r, kind="ExternalInput").ap()
    b1_d = nc.dram_tensor("b1", [L, DFF], f32, kind="ExternalInput").ap()
    w2_d = nc.dram_tensor("w2", [L, DFF, C], f32r, kind="ExternalInput").ap()
    b2_d = nc.dram_tensor("b2", [L, C], f32, kind="ExternalInput").ap()
    wlm_d = nc.dram_tensor("wlm", [C, V], f32r, kind="ExternalInput").ap()
    wqksum_d = nc.dram_tensor("wqksum", [L, 2, 2 * C], f32r, kind="ExternalInput").ap()
    wvsum_d = nc.dram_tensor("wvsum", [L, 2, C], f32r, kind="ExternalInput").ap()
    w1sum_d = nc.dram_tensor("w1sum", [L, 2, DFF], f32r, kind="ExternalInput").ap()
    wlmsum_d = nc.dram_tensor("wlmsum", [2, V], f32r, kind="ExternalInput").ap()
    blm_d = nc.dram_tensor("blm", [V], f32, kind="ExternalInput").ap()
    outT_d = nc.dram_tensor("outT", [V, NTOK], f32, kind="ExternalOutput").ap()

    with tile.TileContext(nc) as tc:
        _build_body(nc, tc, x0T_d, wqkv_d, bqkv_d, wo_d, bo_d, w1_d, b1_d,
                    w2_d, b2_d, wlm_d, blm_d, outT_d,
                    wqksum_d, wvsum_d, w1sum_d, wlmsum_d)
    nc.compile()
    return nc


def _build_body(nc, tc, x0T_d, wqkv_d, bqkv_d, wo_d, bo_d, w1_d, b1_d,
                w2_d, b2_d, wlm_d, blm_d, outT_d,
                wqksum_d, wvsum_d, w1sum_d, wlmsum_d):
    import contextlib
    ctx = contextlib.ExitStack()
    p_const = ctx.enter_context(tc.tile_pool(name="consts", bufs=1))
    p_x = ctx.enter_context(tc.tile_pool(name="x", bufs=1))
    p_xn = ctx.enter_context(tc.tile_pool(name="xn", bufs=1))
    p_qk = ctx.enter_context(tc.tile_pool(name="qk", bufs=2))
    p_v = ctx.enter_context(tc.tile_pool(name="v", bufs=2))
    p_w = ctx.enter_context(tc.tile_pool(name="w", bufs=1))
    p_b = ctx.enter_context(tc.tile_pool(name="b", bufs=2))
    p_tr = ctx.enter_context(tc.tile_pool(name="tr", bufs=2))   # transient 512-wide
    p_at = ctx.enter_context(tc.tile_pool(name="at", bufs=2))   # attc accum
    p_sm = ctx.enter_context(tc.tile_pool(name="sm", bufs=2))   # rec / r_b
    p_e = ctx.enter_context(tc.tile_pool(name="e", bufs=2))
    p_st = ctx.enter_context(tc.tile_pool(name="st", bufs=2))   # LN stats rows
    p_bc = ctx.enter_context(tc.tile_pool(name="bc", bufs=2))   # LN broadcast tiles
    p_ff = ctx.enter_context(tc.tile_pool(name="ff", bufs=2))
    p_out = ctx.enter_context(tc.tile_pool(name="out", bufs=1))
    psum = ctx.enter_context(tc.tile_pool(name="psum", bufs=2, space="PSUM"))

    # ---- constants ----
    stage = p_const.tile([128, 64], f32, tag="stage")

    # m01 [128,512] 0/1 multiplicative causal mask: cols 0:256 keep p<=f
    # (kb0), cols 256:512 keep p<=f-256 i.e. local q offset 128 (kb1)
    m01 = p_const.tile([128, 512], f32, tag="m01")
    nc.vector.memset(m01[:, 0:256], 1.0)
    nc.gpsimd.affine_select(out=m01[:, 0:256], in_=m01[:, 0:256],
                            compare_op=ALU.is_ge, fill=0.0,
                            base=0, pattern=[[1, 256]], channel_multiplier=-1)
    nc.vector.memset(m01[:, 256:512], 1.0)
    nc.gpsimd.affine_select(out=m01[:, 256:512], in_=m01[:, 256:512],
                            compare_op=ALU.is_ge, fill=0.0,
                            base=-128, pattern=[[1, 256]], channel_multiplier=-1)

    onesC = p_const.tile([128, 2], f32r, tag="onesC")   # 1/C for mean matmuls
    nc.vector.memset(stage[:, 0:2], 1.0 / C)
    nc.vector.tensor_copy(onesC[:], stage[:, 0:2])
    onesH = p_const.tile([128, H], f32r, tag="onesH")  # ones columns for V_ext
    nc.vector.memset(stage[:, 8:8 + H], 1.0)
    nc.vector.tensor_copy(onesH[:], stage[:, 8:8 + H])
    eps_t = p_const.tile([1, 1], f32, tag="eps")
    nc.vector.memset(eps_t, EPS)

    blm_t = p_const.tile([V, 1], f32, tag="blm")
    nc.sync.dma_start(out=blm_t, in_=blm_d.rearrange("(v o) -> v o", o=1))
    wlm_t = [p_const.tile([128, V], f32r, tag=f"wlm{kc}", name=f"wlm{kc}")
             for kc in range(KC)]
    for kc in range(KC):
        nc.sync.dma_start(out=wlm_t[kc], in_=wlm_d[kc * 128:(kc + 1) * 128, :])

    # ---- residual stream ----
    x_t = [[p_x.tile([128, 512], f32r, tag=f"x{kc}_{nt}", name=f"x{kc}_{nt}")
            for nt in range(NT)] for kc in range(KC)]
    for kc in range(KC):
        for nt in range(NT):
            nc.sync.dma_start(out=x_t[kc][nt],
                              in_=x0T_d[kc * 128:(kc + 1) * 128,
                                        nt * 512:nt * 512 + 512])

    def layernorm(src_tiles, tagp):
        """scale-standardize: returns xs = x * rs_b as a 3x4 grid of f32r
        tiles; the mean correction mr = mu*rs is left in mr2 row 0 and must
        be folded by the consumer via a K=2 rank-1 matmul with -colsum
        weights (xn = xs - broadcast(mr))."""
        out_tiles = [[p_xn.tile([128, 512], f32r, tag=f"{tagp}{kc}_{nt}",
                                name=f"{tagp}{kc}_{nt}") for nt in range(NT)]
                     for kc in range(KC)]
        for nt in range(NT):
            cols = slice(nt * 512, nt * 512 + 512)
            c0 = nt * 512
            A = p_st.tile([1, 512], f32, tag="lnA", name="lnA")[:]
            Br = p_st.tile([1, 512], f32, tag="lnB", name="lnB")[:]
            Cr = p_st.tile([1, 512], f32, tag="lnC", name="lnC")[:]
            mu_ps = psum.tile([2, 512], f32, tag="pa", name="mu_ps")
            sq_ps = psum.tile([2, 512], f32, tag="pc", name="sq_ps", bufs=3)
            for kc in range(KC):
                nc.tensor.matmul(mu_ps[:], onesC[:], src_tiles[kc][nt][:],
                                 start=(kc == 0), stop=(kc == KC - 1))
            for kc in range(KC):
                xsq = p_tr.tile([128, 512], f32r, tag="tr512", name="xsq")
                nc.scalar.square(xsq[:], src_tiles[kc][nt][:])
                nc.tensor.matmul(sq_ps[:], onesC[:], xsq[:],
                                 start=(kc == 0), stop=(kc == KC - 1))
            nc.scalar.copy(A, mu_ps[0:1, :])
            nc.scalar.copy(Br, sq_ps[0:1, :])
            nc.vector.tensor_mul(Cr, A, A)                       # tmp = mu^2
            nc.vector.tensor_tensor(out=Br, in0=Br, in1=Cr, op=ALU.subtract)
            nc.scalar.activation(Br, Br, AF.Sqrt, bias=eps_t[:], scale=1.0)
            nc.vector.reciprocal_approx_fast(out=Br, in_=Br)     # B = rs
            nc.vector.tensor_mul(Cr, A, Br)                      # C = mr = mu*rs
            rs_b = p_bc.tile([128, 512], f32, tag="rs_b")
            mr_b = p_bc.tile([128, 512], f32, tag="mr_b")
            nc.gpsimd.partition_broadcast(rs_b[:], Br)
            nc.gpsimd.partition_broadcast(mr_b[:], Cr)
            for kc in range(KC):
                t1 = p_tr.tile([128, 512], f32r, tag="tr512", name="lnt1")
                nc.gpsimd.tensor_mul(t1[:], src_tiles[kc][nt][:], rs_b[:])
                nc.vector.tensor_tensor(out=out_tiles[kc][nt][:], in0=t1[:],
                                        in1=mr_b[:], op=ALU.subtract)
        return out_tiles

    for l in range(N_LAYERS):
        # ---- per-layer weights ----
        wqkv_t = [p_w.tile([128, 3 * C], f32r, tag=f"wqkv{kc}", name=f"wqkv{kc}")
                  for kc in range(KC)]
        for kc in range(KC):
            nc.sync.dma_start(out=wqkv_t[kc],
                              in_=wqkv_d[l, kc * 128:(kc + 1) * 128, :])
        bqkv_t = p_b.tile([128, 6], f32, tag="bqkv")
        nc.sync.dma_start(out=bqkv_t,
                          in_=bqkv_d[l].rearrange("(a p) -> p a", p=128))
        wo_t = [p_w.tile([128, C], f32r, tag=f"wo{kc}", name=f"wo{kc}")
                for kc in range(KC)]
        for kc in range(KC):
            nc.sync.dma_start(out=wo_t[kc], in_=wo_d[l, kc * 128:(kc + 1) * 128, :])
        bo_t = p_b.tile([128, 3], f32, tag="bo")
        nc.sync.dma_start(out=bo_t, in_=bo_d[l].rearrange("(a p) -> p a", p=128))
        w1_t = [p_w.tile([128, DFF], f32r, tag=f"w1{kc}", name=f"w1{kc}")
                for kc in range(KC)]
        for kc in range(KC):
            nc.sync.dma_start(out=w1_t[kc], in_=w1_d[l, kc * 128:(kc + 1) * 128, :])
        b1_t = p_b.tile([128, 12], f32, tag="b1")
        nc.sync.dma_start(out=b1_t, in_=b1_d[l].rearrange("(a p) -> p a", p=128))
        w2_t = [p_w.tile([128, C], f32r, tag=f"w2{kc}", name=f"w2k{kc}")
                for kc in range(12)]
        for kc in range(12):
            nc.sync.dma_start(out=w2_t[kc], in_=w2_d[l, kc * 128:(kc + 1) * 128, :])
        b2_t = p_b.tile([128, 3], f32, tag="b2")
        nc.sync.dma_start(out=b2_t, in_=b2_d[l].rearrange("(a p) -> p a", p=128))

        # ---- LN1 ----
        xn = layernorm(x_t, "ln")

        # ---- attention, per pair of batches (QKV/Wo at N=512) ----
        for bp in range(BPC // 2):
            nt = bp  # 512-col tile index == batch pair index
            # Q,K for 2 batches: feature-major [128, 512]
            qk_t = [p_qk.tile([128, 512], f32r, tag=f"qk{oc}", name=f"qk{oc}")
                    for oc in range(6)]
            for oc in range(6):
                qp = psum.tile([128, 512], f32, tag="pa", name="qp")
                for kc in range(KC):
                    nc.tensor.matmul(qp[:], wqkv_t[kc][:, oc * 128:oc * 128 + 128],
                                     xn[kc][nt][:],
                                     start=(kc == 0), stop=(kc == KC - 1))
                nc.scalar.activation(qk_t[oc][:], qp[:], AF.Identity,
                                     bias=bqkv_t[:, oc:oc + 1], scale=1.0)
            attc = [p_at.tile([128, 512], f32r, tag=f"attc{kc}", name=f"attc{kc}")
                    for kc in range(KC)]
            for bi in range(2):
                q0 = bi * 256        # local q offset in the pair
                # V token-major with ones column, 2 chunks of 128 tokens
                vext = [p_v.tile([128, H * (HS + 1)], f32r, tag=f"vext{i}",
                                 name=f"vext{i}") for i in range(2)]
                for i in range(2):
                    vp = psum.tile([128, C], f32, tag="pa", name="vp")
                    tc0 = q0 + i * 128
                    for kc in range(KC):
                        nc.tensor.matmul(vp[:], xn[kc][nt][:, tc0:tc0 + 128],
                                         wqkv_t[kc][:, 2 * C:3 * C],
                                         start=(kc == 0), stop=(kc == KC - 1))
                    vx = vext[i].rearrange("p (h e) -> p h e", h=H)
                    nc.vector.tensor_copy(vx[:, :, 0:HS],
                                          vp[:].rearrange("p (h d) -> p h d", h=H))
                    nc.gpsimd.tensor_copy(out=vx[:, :, HS:HS + 1], in_=onesH[:])

                for h in range(H):
                    qrow = (h % 2) * 64
                    qch, kch = h // 2, 3 + h // 2
                    sp = psum.tile([128, 512], f32, tag="pc", name="sp", bufs=3)
                    qs = qk_t[qch][qrow:qrow + 64, q0:q0 + 256]
                    nc.tensor.matmul(sp[:, 0:256],
                                     qk_t[kch][qrow:qrow + 64, q0:q0 + 128],
                                     qs, start=True, stop=True)
                    nc.tensor.matmul(sp[:, 256:512],
                                     qk_t[kch][qrow:qrow + 64, q0 + 128:q0 + 256],
                                     qs, start=True, stop=True)
                    e_t = p_e.tile([128, 512], f32, tag="e")
                    nc.scalar.activation(e_t[:], sp[:], AF.Exp, bias=0.0,
                                         scale=SCALE)
                    e_m = p_e.tile([128, 512], f32r, tag="em")
                    nc.vector.tensor_mul(e_m[:], e_t[:], m01[:])
                    ap_ = psum.tile([HS + 1, T], f32, tag="pd", name="ap_", bufs=3)
                    nc.tensor.matmul(ap_[:],
                                     vext[0][:, h * (HS + 1):(h + 1) * (HS + 1)],
                                     e_m[:, 0:256], start=True, stop=False)
                    nc.tensor.matmul(ap_[:],
                                     vext[1][:, h * (HS + 1):(h + 1) * (HS + 1)],
                                     e_m[:, 256:512], start=False, stop=True)
                    srow = p_sm.tile([1, T], f32, tag="srow")
                    nc.scalar.copy(srow[:], ap_[HS:HS + 1, :])
                    rec = p_sm.tile([1, T], f32, tag="rec")
                    nc.vector.reciprocal_approx_fast(out=rec[:], in_=srow[:])
                    r_b = p_sm.tile([64, T], f32, tag="r_b")
                    nc.gpsimd.partition_broadcast(r_b[:], rec[:])
                    nc.vector.tensor_mul(
                        attc[h // 2][qrow:qrow + 64, q0:q0 + 256],
                        ap_[0:HS, :], r_b[:])

            # Wo + residual for this batch pair (N=512)
            for oc in range(KC):
                wp = psum.tile([128, 512], f32, tag="pa", name="wp")
                for kc in range(KC):
                    nc.tensor.matmul(wp[:], wo_t[kc][:, oc * 128:oc * 128 + 128],
                                     attc[kc][:], start=(kc == 0),
                                     stop=(kc == KC - 1))
                wsb = p_tr.tile([128, 512], f32, tag="tr512", name="wsb")
                nc.scalar.activation(wsb[:], wp[:], AF.Identity,
                                     bias=bo_t[:, oc:oc + 1], scale=1.0)
                nc.gpsimd.tensor_add(x_t[oc][nt][:], wsb[:], x_t[oc][nt][:])

        # ---- LN2 + FFN (interleaved: each ff1 chunk consumed right away) ----
        h2 = layernorm(x_t, "ln")
        for nt in range(NT):
            cols = slice(nt * 512, nt * 512 + 512)
            fp2 = [psum.tile([128, 512], f32, tag=t, name=f"fp2{t}", bufs=bb)
                   for t, bb in (("pa", 2), ("pc", 3), ("pd", 3))]
            for kc12 in range(12):
                fp1 = psum.tile([128, 512], f32, tag="pd", name="fp1", bufs=3)
                for kc in range(KC):
                    nc.tensor.matmul(fp1[:], w1_t[kc][:, kc12 * 128:kc12 * 128 + 128],
                                     h2[kc][nt][:],
                                     start=(kc == 0), stop=(kc == KC - 1))
                ff1 = p_ff.tile([128, 512], f32r, tag="ff1", name="ff1")
                nc.scalar.activation(ff1[:], fp1[:], AF.Relu,
                                     bias=b1_t[:, kc12:kc12 + 1], scale=1.0)
                for oc in range(KC):
                    nc.tensor.matmul(fp2[oc][:], w2_t[kc12][:, oc * 128:oc * 128 + 128],
                                     ff1[:], start=(kc12 == 0), stop=(kc12 == 11))
            for oc in range(KC):
                fsb = p_tr.tile([128, 512], f32, tag="tr512", name="fsb")
                nc.scalar.activation(fsb[:], fp2[oc][:], AF.Identity,
                                     bias=b2_t[:, oc:oc + 1], scale=1.0)
                nc.vector.tensor_add(x_t[oc][nt][:], fsb[:], x_t[oc][nt][:])

    # ---- final LN + LM head ----
    xf = layernorm(x_t, "ln")
    for nt in range(NT):
        cols = slice(nt * 512, nt * 512 + 512)
        lp = psum.tile([V, 512], f32, tag="pa", name="lp")
        for kc in range(KC):
            nc.tensor.matmul(lp[:], wlm_t[kc][:], xf[kc][nt][:],
                             start=(kc == 0), stop=(kc == KC - 1))
        osb = p_out.tile([V, 512], f32, tag="osb")
        nc.scalar.activation(osb[:], lp[:], AF.Identity, bias=blm_t[:], scale=1.0)
        nc.sync.dma_start(out=outT_d[:, cols], in_=osb[:])

    ctx.close()


def _host_prep(inputs):
    """Fold LN affine params into weights; build per-core input maps."""
    f = lambda k: np.asarray(inputs[k], dtype=np.float32)
    idx = np.asarray(inputs["idx"]).astype(np.int64)
    tok_emb, pos_emb = f("tok_emb"), f("pos_emb")
    Wq, Wk, Wv, Wo = f("Wq"), f("Wk"), f("Wv"), f("Wo")
    bo, W1, b1, W2, b2 = f("bo"), f("W1"), f("b1"), f("W2"), f("b2")
    ln1_g, ln1_b = f("ln1_g"), f("ln1_b")
    ln2_g, ln2_b = f("ln2_g"), f("ln2_b")
    lnf_g, lnf_b = f("lnf_g"), f("lnf_b")
    Wlm, blm = f("Wlm"), f("blm")

    # [L,H,C,HS] -> [L,C,H*HS]
    Wq_all = np.transpose(Wq, (0, 2, 1, 3)).reshape(L, C, C)
    Wk_all = np.transpose(Wk, (0, 2, 1, 3)).reshape(L, C, C)
    Wv_all = np.transpose(Wv, (0, 2, 1, 3)).reshape(L, C, C)

    g1 = ln1_g[:, :, None]
    wqkv = np.concatenate([g1 * Wq_all, g1 * Wk_all, g1 * Wv_all], axis=2)
    def neg_colsum2(w):                      # [L?,C,D] -> [.,2,D] row0=-colsum
        s = -w.sum(axis=-2)
        z = np.zeros_like(s)
        return np.stack([s, z], axis=-2)
    bq = np.einsum("lc,lcd->ld", ln1_b, Wq_all)
    bk = np.einsum("lc,lcd->ld", ln1_b, Wk_all)
    bv = np.einsum("lc,lcd->ld", ln1_b, Wv_all)
    bqkv = np.concatenate([bq, bk], axis=1)
    bo2 = bo + np.einsum("ld,ldc->lc", bv, Wo)       # v-bias folds through Wo
    w1f = ln2_g[:, :, None] * W1
    b1f = b1 + np.einsum("lc,lcd->ld", ln2_b, W1)
    wlmf = lnf_g[:, None] * Wlm
    blmf = blm + lnf_b @ Wlm

    wqksum = neg_colsum2(wqkv[:, :, :2 * C])         # [L,2,768]
    wvsum = neg_colsum2(wqkv[:, :, 2 * C:])          # [L,2,384]
    w1sum = neg_colsum2(w1f)                         # [L,2,1536]
    wlmsum = neg_colsum2(wlmf)                       # [2,65]

    x0 = tok_emb[idx] + pos_emb[None]                # [B,T,C] f32
    in_maps = []
    for c in range(N_CORES):
        x0c = x0[c * BPC:(c + 1) * BPC].reshape(NTOK, C)
        in_maps.append({
            "x0T": np.ascontiguousarray(x0c.T),
            "wqkv": np.ascontiguousarray(wqkv),
            "bqkv": np.ascontiguousarray(bqkv),
            "wo": np.ascontiguousarray(Wo),
            "bo": np.ascontiguousarray(bo2),
            "w1": np.ascontiguousarray(w1f),
            "b1": np.ascontiguousarray(b1f),
            "w2": np.ascontiguousarray(W2),
            "b2": np.ascontiguousarray(b2),
            "wlm": np.ascontiguousarray(wlmf),
            "blm": np.ascontiguousarray(blmf),
            "wqksum": np.ascontiguousarray(wqksum),
            "wvsum": np.ascontiguousarray(wvsum),
            "w1sum": np.ascontiguousarray(w1sum),
            "wlmsum": np.ascontiguousarray(wlmsum),
        })
    return in_maps


def _run(inputs, trace=False):
    if "nc" not in _cache:
        _cache["nc"] = _build_nc()
    nc = _cache["nc"]
    in_maps = _host_prep(inputs)
    res = run_bass_kernel_spmd(nc, in_maps, core_ids=list(range(N_CORES)),
                               trace=trace)
    outs = []
    for c in range(N_CORES):
        outT = res.results[c]["outT"]                 # [V, NTOK]
        outs.append(outT.T.reshape(BPC, T, V))
    logits = np.concatenate(outs, axis=0).astype(np.float32)
    return logits, res


def kernel(**inputs) -> np.ndarray:
    logits, _ = _run(inputs, trace=False)
    return logits
